# revision 1
# baseline (speedup 1.0000x reference)
"""Cosine cross-attention (B=4, L=2048, D=1024, H=16, dh=64, tau=0.07) on 8 trn2 cores.

Sharding: core = b*2 + g  (b in 0..3 data-parallel, g in 0..1 head-group of 8 heads).
Per core everything is computed feature-major ("T" = transposed, [feature, L]):
  QT = l2norm-by-head( wq.T @ xqT + bq )        [512, 2048]  (normalization via
      PE-broadcast of 1/||q|| and in-place DVE multiply)
  KT =                 wk.T @ xkT + bk          [512, 2048]  (its 1/||k||/tau goes
      into the per-partition scale of the exp activation)
  V  natural layout    (xvT.T chunks) @ wv      [2048, 512]  no bias (bv folded
      into a host-side output bias: softmax rows sum to 1)
  per head pair (m) / head (s):   S.T tile = KT_h.T-chunk.T @ QT-block  (K=64,
      auto row-tiled 64x128 so the two heads share the PE array; K itself is
      pre-scaled by rnk/tau so the exp needs no per-partition scale and one
      ACT call covers both heads)
  E.T = exp(S.T)  on ACT, psum->sbuf f32r
  OT  = [V | 1].T @ E.T  accumulated over Lk in PSUM -> row 64 is the softmax Z
  MT  = OT[0:64]/Z  via DVE reciprocal + PE-broadcast + in-place multiply
  OUT.T partial = wo-chunks.T @ MT-chunks       [1024, 2048]
Host: out[b] = (partial_g0 + partial_g1).T + (bo + bv @ Wo.T).

All matmuls run as float32r (tf32-like, 1 cycle/row at N>=256).
"""

import os

# some harnesses pin jax to cpu for the reference; this kernel needs the
# axon/neuron backend, so clear the pin before jax is first imported
if os.environ.get("JAX_PLATFORMS") == "cpu":
    del os.environ["JAX_PLATFORMS"]

import numpy as np

import concourse.bacc as bacc
import concourse.tile as tile
from concourse import mybir
from concourse.bass_utils import run_bass_kernel_spmd

P = 128
L = 2048
D = 1024
DO = 512  # per-core output dims of q/k/v projections (8 heads * 64)
TAU = 0.07
NLB = L // 512   # 4 blocks of 512 along L
NLK = L // 128   # 16 chunks of 128 along L (keys)
NM = DO // P     # 4 dout chunks (head pairs)
NKC = D // P     # 8 contraction chunks for projections

F32 = mybir.dt.float32
F32R = mybir.dt.float32r
BF16 = mybir.dt.bfloat16
EXP = mybir.ActivationFunctionType.Exp
SQRT = mybir.ActivationFunctionType.Sqrt
SQUARE = mybir.ActivationFunctionType.Square
MULT = mybir.AluOpType.mult

_CACHE = {}
VARIANT = None


def _emit(nc, prm, repeat=1, phases="abcd"):
    from contextlib import ExitStack
    with tile.TileContext(nc) as tc:
        if repeat > 1:
            with tc.For_i(0, repeat, 1):
                _emit_body(nc, tc, prm, phases)
        else:
            _emit_body(nc, tc, prm, phases)


def _emit_body(nc, tc, prm, phases="abcd"):
    from contextlib import ExitStack
    with ExitStack() as stack:
        const = stack.enter_context(tc.tile_pool(name="const", bufs=1))
        persist = stack.enter_context(tc.tile_pool(name="persist", bufs=1))
        normp = stack.enter_context(tc.tile_pool(name="normp", bufs=2))

        indt = const.tile([P, 2], F32R, tag="indt")
        nc.sync.dma_start(out=indt[:], in_=prm["indt"][:])
        ones8 = const.tile([P, 8], F32R, tag="ones8")
        nc.sync.dma_start(out=ones8[:], in_=prm["ones8"][:])
        selq = const.tile([8, NM, P], F32R, tag="selq")
        nc.sync.dma_start(out=selq[:], in_=prm["selq"][:])
        selz = const.tile([8, 8, 64], F32R, tag="selz")
        nc.sync.dma_start(out=selz[:], in_=prm["selz"][:])
        bq_t = const.tile([P, NM], F32, tag="bq")
        bk_t = const.tile([P, NM], F32, tag="bk")
        for m in range(NM):
            nc.sync.dma_start(out=bq_t[:, m], in_=prm["bq"][m * P:(m + 1) * P])
            nc.sync.dma_start(out=bk_t[:, m], in_=prm["bk"][m * P:(m + 1) * P])

        qt = [persist.tile([P, L], F32R, tag=f"qt{m}", name=f"qt{m}") for m in range(NM)]
        kt = [persist.tile([P, L], F32R, tag=f"kt{m}", name=f"kt{m}") for m in range(NM)]
        vg_all = persist.tile([P, NLK, 8, 65], F32R, tag="vg_all")
        vg = [vg_all[:, i] for i in range(NLK)]
        nq_all = persist.tile([8, L], F32R, tag="nq_all")
        nk_all = persist.tile([8, L], F32R, tag="nk_all")

        # ---------------- Phase A: projections ----------------
        with tc.tile_pool(name="wp", bufs=1) as wp, \
             tc.tile_pool(name="xp", bufs=3) as xp, \
             tc.tile_pool(name="sqp", bufs=1) as sqp, \
             tc.tile_pool(name="psA", bufs=1, space="PSUM") as psA, \
             tc.tile_pool(name="psN", bufs=1, space="PSUM") as psN, \
             tc.tile_pool(name="psBC", bufs=2, space="PSUM") as psBC:

            for kind in (("q", "k", "v") if "a" in phases else ()):
                w_d = prm["w" + kind]
                x_d = prm["x" + kind]
                wt = []
                for kc in range(NKC):
                    w_t = wp.tile([P, DO], F32R, tag=f"w{kc}")
                    nc.sync.dma_start(out=w_t[:], in_=w_d[kc * P:(kc + 1) * P, :])
                    wt.append(w_t)
                for lb in range(NLB):
                    pas = [psA.tile([P, 512], F32, tag=f"pa{j}", name=f"pa{j}") for j in range(NM)]
                    for kc2 in range(NKC // 2):
                        # paired-chunk load: one DMA brings two contraction chunks
                        x_t = xp.tile([P, 2, 512], F32R, tag="x")
                        nc.sync.dma_start(
                            out=x_t[:],
                            in_=x_d[2 * kc2 * P:(2 * kc2 + 2) * P,
                                    lb * 512:(lb + 1) * 512].rearrange(
                                        "(two p) i -> p two i", two=2))
                        for half in range(2):
                            kc = 2 * kc2 + half
                            xv = x_t[:, half, :]
                            if kind == "v":
                                for j in range(NM):
                                    nc.tensor.matmul(
                                        pas[j][:], lhsT=xv[:, j * P:(j + 1) * P], rhs=wt[kc][:],
                                        start=(kc == 0), stop=(kc == NKC - 1))
                            else:
                                for m in range(NM):
                                    nc.tensor.matmul(
                                        pas[m][:], lhsT=wt[kc][:, m * P:(m + 1) * P], rhs=xv,
                                        start=(kc == 0), stop=(kc == NKC - 1))
                    if kind == "v":
                        for j in range(NM):
                            lc = lb * 4 + j
                            nc.vector.tensor_copy(
                                out=vg[lc][:, :, 0:64],
                                in_=pas[j][:].rearrange("p (h d) -> p h d", h=8))
                            nc.vector.tensor_copy(out=vg[lc][:, :, 64], in_=ones8[:])
                    else:
                        b_t = bq_t if kind == "q" else bk_t
                        n_all = nq_all if kind == "q" else nk_all
                        for m in range(NM):
                            sl = slice(lb * 512, (lb + 1) * 512)
                            blk = (qt if kind == "q" else kt)[m][:, sl]
                            nc.vector.tensor_scalar_add(
                                out=blk, in0=pas[m][:], scalar1=b_t[:, m:m + 1])
                            sq_t = sqp.tile([P, 512], F32R, tag="sq")
                            # (x+b)^2 in one ACT op -- ScalarE is idle in phase A
                            nc.scalar.activation(out=sq_t[:], in_=pas[m][:], func=SQUARE,
                                                 bias=b_t[:, m:m + 1])
                            nqp = psN.tile([2, 512], F32, tag="nq")
                            nc.tensor.matmul(nqp[:], lhsT=indt[:], rhs=sq_t[:],
                                             start=True, stop=True)
                            nqb = normp.tile([2, 512], F32R, tag="nqb", bufs=1)
                            nc.vector.tensor_copy(out=nqb[:], in_=nqp[:])
                            nc.sync.dma_start(
                                out=n_all[2 * m:2 * m + 2, lb * 512:(lb + 1) * 512],
                                in_=nqb[:])

            # ---------------- Phase B: norms ----------------
            if "b" in phases:
                _emit_norms(nc, tc, normp, psBC, qt, kt, nq_all, nk_all, selq)

        # prefetch out-projection weights so phase D starts immediately
        wop = stack.enter_context(tc.tile_pool(name="wop", bufs=1))
        wot = []
        for kc in range(NM):
            w_t = wop.tile([P, D], F32R, tag=f"wo{kc}", name=f"wo{kc}")
            nc.sync.dma_start(out=w_t[:], in_=prm["wo"][kc * P:(kc + 1) * P, :])
            wot.append(w_t)

        # ---------------- Phase C: attention ----------------
        mtp = stack.enter_context(tc.tile_pool(name="mtp", bufs=1))
        mt = [mtp.tile([P, L], F32R, tag=f"mt{m}", name=f"mt{m}") for m in range(NM)]
        with tc.tile_pool(name="psS", bufs=2, space="PSUM") as psS, \
             tc.tile_pool(name="psOT", bufs=2, space="PSUM") as psOT, \
             tc.tile_pool(name="etp", bufs=4) as etp:
            for m in range(NM if "c" in phases else 0):
                zpack = normp.tile([8, 512], F32R, tag="zp", bufs=1)
                for lq in range(NLB):
                    ot0 = psOT.tile([65, 512], F32, tag="ot0")
                    ot1 = psOT.tile([65, 512], F32, tag="ot1")
                    for lk in range(NLK):
                        pss = psS.tile([P, 1024], F32, tag="pss")
                        # the two heads run in opposite PE array halves
                        for s in range(2):
                            base = s * 64
                            nc.tensor.matmul(
                                pss[:, s * 512:(s + 1) * 512],
                                lhsT=kt[m][base:base + 64, lk * P:(lk + 1) * P],
                                rhs=qt[m][base:base + 64, lq * 512:(lq + 1) * 512],
                                start=True, stop=True)
                        et = etp.tile([P, 1024], F32R, tag="et")
                        nc.scalar.activation(out=et[:], in_=pss[:], func=EXP)
                        _emit_pv(nc, vg, et, lk, ot0, ot1, m)
                    for s, ot in enumerate((ot0, ot1)):
                        nc.vector.tensor_copy(
                            out=mt[m][s * 64:s * 64 + 64, lq * 512:(lq + 1) * 512],
                            in_=ot[0:64, :])
                        zb = normp.tile([1, 512], F32R, tag="zb", bufs=1)
                        nc.vector.tensor_copy(out=zb[:], in_=ot[64:65, :])
                        r = s * 4 + lq
                        nc.sync.dma_start(out=zpack[r:r + 1, :], in_=zb[:])
                rz = zpack
                with nc.allow_low_precision(reason="f32r reciprocal, tf32 rounding is fine here"):
                    nc.vector.reciprocal(out=rz[:], in_=zpack[:])
                for s in range(2):
                    for lq in range(NLB):
                        r = s * 4 + lq
                        # borrow a pss slot for the Z broadcast (the score
                        # pipeline drains at the m boundary anyway)
                        bc = psS.tile([P, 1024], F32, tag="pss")
                        nc.tensor.matmul(bc[0:64, 0:512], lhsT=selz[:, r, :], rhs=rz[:],
                                         start=True, stop=True)
                        blk = mt[m][s * 64:s * 64 + 64, lq * 512:(lq + 1) * 512]
                        nc.vector.tensor_tensor(out=blk, in0=blk, in1=bc[0:64, 0:512],
                                                op=MULT)

        # ---------------- Phase D: output projection ----------------
        if "d" not in phases:
            ob0 = normp.tile([P, 512], F32, tag="dummyout")
            nc.vector.memset(ob0[:], 0.0)
            nc.sync.dma_start(out=prm["out_t"][0:P, 0:512], in_=ob0[:])
            return
        with tc.tile_pool(name="obp", bufs=4) as obp, \
             tc.tile_pool(name="psD", bufs=4, space="PSUM") as psD:
            for mo in range(D // P):
                for lb in range(NLB):
                    pd = psD.tile([P, 512], F32, tag="pd")
                    for kc in range(NM):
                        nc.tensor.matmul(pd[:], lhsT=wot[kc][:, mo * P:(mo + 1) * P],
                                         rhs=mt[kc][:, lb * 512:(lb + 1) * 512],
                                         start=(kc == 0), stop=(kc == NM - 1))
                    ob = obp.tile([P, 512], F32, tag="ob")
                    nc.vector.tensor_copy(out=ob[:], in_=pd[:])
                    nc.sync.dma_start(
                        out=prm["out_t"][mo * P:(mo + 1) * P, lb * 512:(lb + 1) * 512],
                        in_=ob[:])



def build_nc(repeat=1, phases="abcd"):
    key = (repeat, phases)
    if key in _CACHE:
        return _CACHE[key]
    nc = bacc.Bacc("TRN2", target_bir_lowering=False, debug=False, num_devices=8)
    prm = {}
    for name in ("xq", "xk", "xv"):
        prm[name] = nc.declare_dram_parameter(name, [D, L], F32R, isOutput=False)
    for name in ("wq", "wk", "wv"):
        prm[name] = nc.declare_dram_parameter(name, [D, DO], F32R, isOutput=False)
    prm["wo"] = nc.declare_dram_parameter("wo", [DO, D], F32R, isOutput=False)
    prm["bq"] = nc.declare_dram_parameter("bq", [DO], F32, isOutput=False)
    prm["bk"] = nc.declare_dram_parameter("bk", [DO], F32, isOutput=False)
    prm["indt"] = nc.declare_dram_parameter("indt", [P, 2], F32R, isOutput=False)
    prm["ones8"] = nc.declare_dram_parameter("ones8", [P, 8], F32R, isOutput=False)
    prm["selq"] = nc.declare_dram_parameter("selq", [8, NM, P], F32R, isOutput=False)
    prm["selz"] = nc.declare_dram_parameter("selz", [8, 8, 64], F32R, isOutput=False)
    prm["out_t"] = nc.declare_dram_parameter("out_t", [D, L], F32, isOutput=True)
    _emit(nc, prm, repeat=repeat, phases=phases)
    nc.compile()
    _CACHE[key] = nc
    return nc


def make_in_maps(q, k, v, Wq, bq, Wk, bk, Wv, bv, Wo, bo):
    B = q.shape[0]
    f32 = np.float32

    indt = np.zeros((P, 2), f32)
    indt[0:64, 0] = 1.0
    indt[64:128, 1] = 1.0
    ones8 = np.ones((P, 8), f32)
    selq = np.zeros((8, NM, P), f32)
    for m in range(NM):
        for j in range(P):
            selq[2 * m + j // 64, m, j] = 1.0
    selz = np.zeros((8, 8, 64), f32)
    for r in range(8):
        selz[r, r, :] = 1.0

    in_maps = []
    for b in range(B):
        for g in range(2):
            sl = slice(g * DO, (g + 1) * DO)
            in_maps.append({
                "xq": np.ascontiguousarray(q[b].T.astype(f32)),
                "xk": np.ascontiguousarray(k[b].T.astype(f32)),
                "xv": np.ascontiguousarray(v[b].T.astype(f32)),
                "wq": np.ascontiguousarray(Wq[sl, :].T.astype(f32)),
                "wk": np.ascontiguousarray(Wk[sl, :].T.astype(f32)),
                "wv": np.ascontiguousarray(Wv[sl, :].T.astype(f32)),
                "wo": np.ascontiguousarray(Wo[:, sl].T.astype(f32)),
                "bq": np.ascontiguousarray(bq[sl].astype(f32)),
                "bk": np.ascontiguousarray(bk[sl].astype(f32)),
                "indt": indt, "ones8": ones8, "selq": selq, "selz": selz,
            })
    return in_maps


def assemble(results, bv, Wo, bo):
    B = len(results) // 2
    bias = (bo + bv @ Wo.T).astype(np.float32)
    outs = []
    for b in range(B):
        part = results[2 * b]["out_t"] + results[2 * b + 1]["out_t"]
        outs.append(part.T + bias)
    return np.stack(outs).astype(np.float32)


def kernel(q, k, v, Wq, bq, Wk, bk, Wv, bv, Wo, bo):
    q, k, v = (np.asarray(t, np.float32) for t in (q, k, v))
    Wq, bq, Wk, bk, Wv, bv, Wo, bo = (
        np.asarray(t, np.float32) for t in (Wq, bq, Wk, bk, Wv, bv, Wo, bo))
    nc = build_nc()
    in_maps = make_in_maps(q, k, v, Wq, bq, Wk, bk, Wv, bv, Wo, bo)
    last_err = None
    for attempt in range(3):
        try:
            res = run_bass_kernel_spmd(nc, in_maps, core_ids=list(range(8)))
            return assemble(res.results, bv, Wo, bo)
        except Exception as e:  # transient NRT device errors: retry
            last_err = e
            import time as _time
            _time.sleep(2.0)
    raise last_err


def _emit_norms(nc, tc, normp, psBC, qt, kt, nq_all, nk_all, selq):
    with nc.allow_low_precision(reason="f32r norm chain, tf32 rounding fine"):
        nc.scalar.activation(out=nq_all[:], in_=nq_all[:], func=SQRT)
        nc.vector.tensor_scalar_max(out=nq_all[:], in0=nq_all[:], scalar1=1e-12)
        nc.vector.reciprocal(out=nq_all[:], in_=nq_all[:])
        nc.scalar.activation(out=nk_all[:], in_=nk_all[:], func=SQRT)
        # clamp at eps, then fold the softmax temperature into k's norm
        nc.vector.tensor_scalar_max(out=nk_all[:], in0=nk_all[:], scalar1=1e-12)
        nc.vector.tensor_scalar_mul(out=nk_all[:], in0=nk_all[:], scalar1=TAU)
        nc.vector.reciprocal(out=nk_all[:], in_=nk_all[:])

    # normalize Q and K in place via PE-broadcast of the row pair
    for which, r_all in (("q", nq_all), ("k", nk_all)):
        for m in range(NM):
            for lb in range(NLB):
                sl = slice(lb * 512, (lb + 1) * 512)
                bc = psBC.tile([P, 512], F32, tag="bcq")
                nc.tensor.matmul(bc[:], lhsT=selq[:, m, :], rhs=r_all[:, sl],
                                 start=True, stop=True)
                blk = (qt if which == "q" else kt)[m][:, sl]
                nc.vector.tensor_tensor(out=blk, in0=blk, in1=bc[:], op=MULT)


def _emit_pv(nc, vg, et, lk, ot0, ot1, m):
    nc.tensor.matmul(ot0[:], lhsT=vg[lk][:, 2 * m, :], rhs=et[:, 0:512],
                     start=(lk == 0), stop=(lk == NLK - 1), skip_group_check=True)
    nc.tensor.matmul(ot1[:], lhsT=vg[lk][:, 2 * m + 1, :], rhs=et[:, 512:1024],
                     start=(lk == 0), stop=(lk == NLK - 1), skip_group_check=True)



# revision 41
# speedup vs baseline: 1.3080x; 1.3080x over previous
"""Cosine cross-attention (B=4, L=2048, D=1024, H=16, dh=64, tau=0.07) on 8 trn2 cores.

Sharding: core = b*2 + g  (b in 0..3 data-parallel, g in 0..1 head-group of 8 heads).

Engine-overlap-oriented structure (v2):
  order: V-proj -> K-proj -> Q-proj -> norm factors -> apply(m=0) -> attention,
  with norm-applies for m>=1, the out-projection, and all broadcast work
  streamed INTO the attention loop so the ACT exp pipeline never waits.

  - One ACT table for everything (natural_log_exp_and_others):
    softmax exp, and 1/||x|| = exp(-0.5*ln(ss)); squares are done on DVE.
  - All partition-broadcasts (q/k norm rows, softmax 1/Z rows) run on the
    otherwise-idle GpSimd engine (partition_broadcast), not the PE.
  - Z: DVE reciprocal of the PSUM ones-row, GpSimd broadcast, fused
    multiply into mt during the PSUM->SBUF move. No DMA, no PE.
  - Attention is lq-outer / m-inner; out-projection chunks for lq run inside
    the (lq+1, m=0) group's exp stream via a deferred-work queue.
  - Bulk x loads are big SWDGE transfers on SP; weights + output stores are
    issued from GpSimd. Nothing contends with ACT/PE/DVE.
  - bf16 storage for kt/vg/et/mt/wo; f32 PSUM accumulation everywhere.
"""

import os

# some harnesses pin jax to cpu for the reference; this kernel needs the
# axon/neuron backend, so clear the pin before jax is first imported
if os.environ.get("JAX_PLATFORMS") == "cpu":
    del os.environ["JAX_PLATFORMS"]

import math

import numpy as np

import concourse.bacc as bacc
import concourse.tile as tile
from concourse import mybir
from concourse.bass_utils import run_bass_kernel_spmd

P = 128
L = 2048
D = 1024
DO = 512  # per-core output dims of q/k/v projections (8 heads * 64)
TAU = 0.07
NLB = L // 512   # 4 blocks of 512 along L
NLK = L // 128   # 16 chunks of 128 along L (keys)
NM = DO // P     # 4 dout chunks (head pairs)
NKC = D // P     # 8 contraction chunks for projections

F32 = mybir.dt.float32
F32R = mybir.dt.float32r
BF16 = mybir.dt.bfloat16
EXP = mybir.ActivationFunctionType.Exp
LN = mybir.ActivationFunctionType.Ln
MULT = mybir.AluOpType.mult
LNTAUINV = -math.log(TAU)  # fold 1/tau into k's norm factor

_CACHE = {}


def _emit(nc, prm, repeat=1, phases="pcd"):
    with tile.TileContext(nc) as tc:
        if repeat > 1:
            with tc.For_i(0, repeat, 1):
                _emit_body(nc, tc, prm, phases)
        else:
            _emit_body(nc, tc, prm, phases)


def _emit_body(nc, tc, prm, phases="pcd"):
    from contextlib import ExitStack
    with ExitStack() as stack:
        const = stack.enter_context(tc.tile_pool(name="const", bufs=1))
        persist = stack.enter_context(tc.tile_pool(name="persist", bufs=1))

        # ---- first x block loads lead the SP queue (v-proj gates on them) ----
        xp = stack.enter_context(tc.tile_pool(name="xp", bufs=3))

        def load_x(name, lb):
            sl = slice(lb * 512, (lb + 1) * 512)
            x0 = xp.tile([P, NKC // 2, 512], F32R, tag="x", name="x0")
            nc.sync.dma_start(
                out=x0[:],
                in_=prm[name][0:512, sl].rearrange("(c p) i -> p c i", c=4))
            x1 = xp.tile([P, NKC // 2, 512], F32R, tag="x", name="x1")
            nc.sync.dma_start(
                out=x1[:],
                in_=prm[name][512:1024, sl].rearrange("(c p) i -> p c i", c=4))
            return x0, x1

        xv_first = load_x("xv", 0)

        # ---- constants ----
        seln = const.tile([P, NM, P], F32R, tag="seln")
        nc.sync.dma_start(out=seln[:], in_=prm["seln"][:])
        selbc = const.tile([P, NM, P], F32R, tag="selbc")
        nc.sync.dma_start(out=selbc[:], in_=prm["selbc"][:])
        bq_t = const.tile([P, NM], F32, tag="bq")
        nc.sync.dma_start(out=bq_t[:], in_=prm["bqp"][:])
        bk_t = const.tile([P, NM], F32, tag="bk")
        nc.sync.dma_start(out=bk_t[:], in_=prm["bkp"][:])
        # col 0: ln-eps bias, col 1: ln(1/tau) (k's exponent bias)
        actc = const.tile([P, 2], F32, tag="actc")
        nc.sync.dma_start(out=actc[:], in_=prm["actc"][:])
        ones8 = const.tile([P, 8], BF16, tag="ones8")
        nc.sync.dma_start(out=ones8[:], in_=prm["ones8"][:])

        # ---- persistent tensors ----
        qt = [persist.tile([P, L], BF16, tag=f"qt{m}", name=f"qt{m}") for m in range(NM)]
        kt = [persist.tile([P, L], BF16, tag=f"kt{m}", name=f"kt{m}") for m in range(NM)]
        mt = [persist.tile([P, L], BF16, tag=f"mt{m}", name=f"mt{m}") for m in range(NM)]
        vg_all = persist.tile([P, NLK, 8, 65], BF16, tag="vg_all")
        nsq = {"q": persist.tile([P, L], F32R, tag="nsq_q", name="nsq_q"),
               "k": persist.tile([P, L], F32R, tag="nsq_k", name="nsq_k")}
        wot = persist.tile([P, NM, D], BF16, tag="wot")

        wp = stack.enter_context(tc.tile_pool(name="wp", bufs=2))
        sqp = stack.enter_context(tc.tile_pool(name="sqp", bufs=8))

        def load_w(name):
            w_t = wp.tile([P, NKC, DO], F32R, tag="w", name=f"w_{name}")
            nc.gpsimd.dma_start(
                out=w_t[:],
                in_=prm[name][:].rearrange("(c p) i -> p c i", c=NKC))
            return w_t

        wv_t = load_w("wv")
        wk_t = load_w("wk")

        # ---------------- V projection (natural layout) ----------------
        with tc.tile_pool(name="psV", bufs=2, space="PSUM") as psV:
            for lb in range(NLB if "p" in phases else 0):
                x0, x1 = xv_first if lb == 0 else load_x("xv", lb)
                for j in range(4):
                    pav = psV.tile([P, 512], F32, tag="pav")
                    for c8 in range(NKC):
                        xt = (x0 if c8 < 4 else x1)
                        nc.tensor.matmul(
                            pav[:],
                            lhsT=xt[:, c8 % 4, j * P:(j + 1) * P],
                            rhs=wv_t[:, c8, :],
                            start=(c8 == 0), stop=(c8 == NKC - 1))
                    lc = lb * 4 + j
                    nc.vector.tensor_copy(
                        out=vg_all[:, lc, :, 0:64],
                        in_=pav[:].rearrange("p (h d) -> p h d", h=8))
                    nc.vector.tensor_copy(out=vg_all[:, lc, :, 64],
                                          in_=ones8[:])
        wq_t = load_w("wq")  # reuses wv's slot; hidden behind k-projection
        nc.gpsimd.dma_start(
            out=wot[:],
            in_=prm["wo"][:].rearrange("(c p) i -> p c i", c=NM))

        # ---------------- Q/K projections + norm factors ----------------
        # Emission primitives shared by the pre-attention path (dedicated
        # psum banks, kc-outer, epilogues deferred one block so the PE never
        # waits on a DVE round trip) and the streamed-q path (small items
        # borrowing PV-pool slots inside the attention loop).
        def proj_mms_m(kind, lb, m, x0, x1, pa):
            w_t = wk_t if kind == "k" else wq_t
            for c8 in range(NKC):
                xt = (x0 if c8 < 4 else x1)
                nc.tensor.matmul(
                    pa[:], lhsT=w_t[:, c8, m * P:(m + 1) * P],
                    rhs=xt[:, c8 % 4, :],
                    start=(c8 == 0), stop=(c8 == NKC - 1))

        def bias_sq_m(kind, lb, m, pa):
            """bias-add into qt/kt (bf16/f32r) + squares; returns sq tile."""
            b_t = bq_t if kind == "q" else bk_t
            dst = qt if kind == "q" else kt
            blk = dst[m][:, slice(lb * 512, (lb + 1) * 512)]
            nc.vector.tensor_scalar_add(out=blk, in0=pa[:],
                                        scalar1=b_t[:, m:m + 1])
            sq_t = sqp.tile([P, 512], F32R, tag="sq")
            nc.vector.tensor_tensor(out=sq_t[:], in0=blk, in1=blk, op=MULT)
            return sq_t

        def nq_mms(kind, lb, sqs, psn):
            """Head-pair square-sums: 4 accumulated row-select matmuls write
            ALL 128 partitions of psn (zeros elsewhere -> no junk rows)."""
            sl = slice(lb * 512, (lb + 1) * 512)
            for m, sq_t in enumerate(sqs):
                nc.tensor.matmul(psn[:], lhsT=seln[:, m, :], rhs=sq_t[:],
                                 start=(m == 0), stop=(m == NM - 1))
            nc.vector.tensor_copy(out=nsq[kind][:, sl], in_=psn[:])

        def emit_norms(kind, cols):
            # 1/||x|| = exp(-0.5*ln(ss + eps)); ln(1/tau) folded into k's bias
            nb = nsq[kind][:, cols]
            with nc.allow_low_precision(reason="norms via ln/exp"):
                nc.scalar.activation(out=nb, in_=nb, func=LN,
                                     bias=actc[:, 0:1])
                if kind == "q":
                    nc.scalar.activation(out=nb, in_=nb, func=EXP, scale=-0.5)
                else:
                    nc.scalar.activation(out=nb, in_=nb, func=EXP, scale=-0.5,
                                         bias=actc[:, 1:2])

        def emit_apply(kind, lb, m, bc):
            sl = slice(lb * 512, (lb + 1) * 512)
            nc.tensor.matmul(bc[:], lhsT=selbc[:, m, :],
                             rhs=nsq[kind][:, sl], start=True, stop=True)
            blk = (qt if kind == "q" else kt)[m][:, sl]
            nc.vector.tensor_tensor(out=blk, in0=blk, in1=bc[:], op=MULT)

        # pre-attention: K fully + Q's first block, software-pipelined so
        # block i's norm/apply matmuls run behind block i+1's projections.
        with tc.tile_pool(name="psA", bufs=1, space="PSUM") as psA, \
             tc.tile_pool(name="psN", bufs=2, space="PSUM") as psN:
            sections = ([("k", lb) for lb in range(NLB)] + [("q", 0)]
                        if "p" in phases else [])
            deferred = None
            for kind, lb in sections + [(None, None)]:
                if kind is not None:
                    x0, x1 = load_x("x" + kind, lb)
                    pas = [psA.tile([P, 512], F32, tag=f"pa{m}", name="pa")
                           for m in range(NM)]
                    for c8 in range(NKC):
                        xt = (x0 if c8 < 4 else x1)
                        for m in range(NM):
                            nc.tensor.matmul(
                                pas[m][:],
                                lhsT=(wk_t if kind == "k" else wq_t)[
                                    :, c8, m * P:(m + 1) * P],
                                rhs=xt[:, c8 % 4, :],
                                start=(c8 == 0), stop=(c8 == NKC - 1))
                    sqs = [bias_sq_m(kind, lb, m, pas[m]) for m in range(NM)]
                if deferred is not None:
                    dkind, dlb, dsqs = deferred
                    psn = psN.tile([P, 512], F32, tag="psn", name="psn")
                    nq_mms(dkind, dlb, dsqs, psn)
                    emit_norms(dkind, slice(dlb * 512, (dlb + 1) * 512))
                    for m in range(NM):
                        bc = psN.tile([P, 512], F32, tag="psn", name="bc")
                        emit_apply(dkind, dlb, m, bc)
                deferred = (kind, lb, sqs) if kind is not None else None

        # ---------------- attention + deferred work ----------------
        zbp = stack.enter_context(tc.tile_pool(name="zbp", bufs=2))

        with tc.tile_pool(name="psS", bufs=2, space="PSUM") as psS, \
             tc.tile_pool(name="psOT", bufs=2, space="PSUM") as psOT, \
             tc.tile_pool(name="etp", bufs=4) as etp, \
             tc.tile_pool(name="zrp", bufs=2) as zrp, \
             tc.tile_pool(name="obp", bufs=2) as obp:

            def emit_epilogue(m, lq, ot0, ot1):
                """mt[m] = OT[0:64] * broadcast(1/Z) (ones-row), bf16 out."""
                sl = slice(lq * 512, (lq + 1) * 512)
                zr0 = zrp.tile([1, 512], F32R, tag="zr", name="zr0")
                zr1 = zrp.tile([1, 512], F32R, tag="zr", name="zr1")
                with nc.allow_low_precision(reason="f32r reciprocal of Z"):
                    nc.vector.reciprocal(out=zr0[:], in_=ot0[64:65, :])
                    nc.vector.reciprocal(out=zr1[:], in_=ot1[64:65, :])
                zbe = zbp.tile([64, 1024], F32R, tag="zbe", name="zbe")
                nc.gpsimd.partition_broadcast(zbe[:, 0:512], zr0[:], channels=64)
                nc.gpsimd.partition_broadcast(zbe[:, 512:1024], zr1[:], channels=64)
                nc.vector.tensor_tensor(out=mt[m][0:64, sl], in0=ot0[0:64, :],
                                        in1=zbe[:, 0:512], op=MULT)
                nc.vector.tensor_tensor(out=mt[m][64:128, sl], in0=ot1[0:64, :],
                                        in1=zbe[:, 512:1024], op=MULT)

            def emit_oproj_chunk(lq, mo):
                """One 128-row chunk of the out-projection for lq's block.
                Uses the PV accumulator pool so the score->exp pipeline's
                double buffer is never disturbed."""
                sl = slice(lq * 512, (lq + 1) * 512)
                pd = psOT.tile([P, 512], F32, tag=("ot0" if mo % 2 == 0 else "ot1"),
                               name="pd")
                for kc in range(NM):
                    nc.tensor.matmul(pd[:], lhsT=wot[:, kc, mo * P:(mo + 1) * P],
                                     rhs=mt[kc][:, sl],
                                     start=(kc == 0), stop=(kc == NM - 1))
                ob = obp.tile([P, 512], F32, tag="ob")
                nc.vector.tensor_copy(out=ob[:], in_=pd[:])
                nc.gpsimd.dma_start(
                    out=prm["out_t"][mo * P:(mo + 1) * P, sl], in_=ob[:])

            _att_slot = [0]

            def alloc_att():
                _att_slot[0] ^= 1
                return psOT.tile([P, 512], F32,
                                 tag=("ot0" if _att_slot[0] else "ot1"),
                                 name="qw")

            # deferred work, popped one item per lk-iteration (lk >= 5).
            # Q-projection blocks 1..3 stream into the early attention groups
            # (they are only needed by the matching lq groups, much later).
            # Their Ln/Exp norm chain is batched into one op pair over the
            # contiguous columns 512:2048 to pay the ACT table switch once.
            pending = []
            if "p" in phases:
                qstate = {lb: {} for lb in range(1, NLB)}
                # x loads self-pace on xp slot availability; issue the first
                # two up front so their transfers hide under the early groups
                qstate[1]["x"] = load_x("xq", 1)
                qstate[2]["x"] = load_x("xq", 2)

                def q_proj(lb, m):
                    st = qstate[lb]
                    pa = alloc_att()
                    proj_mms_m("q", lb, m, st["x"][0], st["x"][1], pa)
                    st.setdefault("sqs", []).append(bias_sq_m("q", lb, m, pa))

                def q_nq(lb):
                    nq_mms("q", lb, qstate[lb]["sqs"], alloc_att())

                for lb in range(1, NLB):
                    for m in range(NM):
                        pending.append(lambda lb=lb, m=m: q_proj(lb, m))
                    pending.append(lambda lb=lb: q_nq(lb))
                    if lb == 1:
                        pending.append(
                            lambda: qstate[3].__setitem__("x", load_x("xq", 3)))
                pending.append(lambda: emit_norms("q", slice(512, 2048)))
                for lb in range(1, NLB):
                    for m in range(NM):
                        pending.append(
                            lambda lb=lb, m=m: emit_apply("q", lb, m,
                                                          alloc_att()))

            groups = [(lq, m) for lq in range(NLB) for m in range(NM)]
            if "c" not in phases:
                groups = []
            for lq, m in groups:
                ot0 = psOT.tile([65, 512], F32, tag="ot0")
                ot1 = psOT.tile([65, 512], F32, tag="ot1")
                for lk in range(NLK):
                    pss = psS.tile([P, 1024], F32, tag="pss", name="pss")
                    for s in range(2):
                        base = s * 64
                        nc.tensor.matmul(
                            pss[:, s * 512:(s + 1) * 512],
                            lhsT=kt[m][base:base + 64, lk * P:(lk + 1) * P],
                            rhs=qt[m][base:base + 64, lq * 512:(lq + 1) * 512],
                            start=True, stop=True)
                    if lk >= 5 and pending:
                        pending.pop(0)()
                    et = etp.tile([P, 1024], BF16, tag="et")
                    nc.scalar.activation(out=et[:], in_=pss[:], func=EXP)
                    nc.tensor.matmul(ot0[:], lhsT=vg_all[:, lk, 2 * m, :],
                                     rhs=et[:, 0:512],
                                     start=(lk == 0), stop=(lk == NLK - 1),
                                     skip_group_check=True)
                    nc.tensor.matmul(ot1[:], lhsT=vg_all[:, lk, 2 * m + 1, :],
                                     rhs=et[:, 512:1024],
                                     start=(lk == 0), stop=(lk == NLK - 1),
                                     skip_group_check=True)
                emit_epilogue(m, lq, ot0, ot1)
                if m == NM - 1 and "d" in phases:
                    for mo in range(D // P):
                        pending.append(
                            lambda lq=lq, mo=mo: emit_oproj_chunk(lq, mo))
            # drain remaining deferred work
            for fn in pending:
                fn()
            if "c" not in phases:
                ob0 = obp.tile([P, 512], F32, tag="ob")
                nc.vector.memset(ob0[:], 0.0)
                nc.gpsimd.dma_start(out=prm["out_t"][0:P, 0:512], in_=ob0[:])


def build_nc(repeat=1, phases="pcd"):
    key = (repeat, phases)
    if key in _CACHE:
        return _CACHE[key]
    nc = bacc.Bacc("TRN2", target_bir_lowering=False, debug=False, num_devices=8)
    prm = {}
    for name in ("xq", "xk", "xv"):
        prm[name] = nc.declare_dram_parameter(name, [D, L], F32R, isOutput=False)
    for name in ("wq", "wk", "wv"):
        prm[name] = nc.declare_dram_parameter(name, [D, DO], F32R, isOutput=False)
    prm["wo"] = nc.declare_dram_parameter("wo", [DO, D], BF16, isOutput=False)
    prm["bqp"] = nc.declare_dram_parameter("bqp", [P, NM], F32, isOutput=False)
    prm["bkp"] = nc.declare_dram_parameter("bkp", [P, NM], F32, isOutput=False)
    prm["seln"] = nc.declare_dram_parameter("seln", [P, NM, P], F32R,
                                            isOutput=False)
    prm["selbc"] = nc.declare_dram_parameter("selbc", [P, NM, P], F32R,
                                             isOutput=False)
    prm["actc"] = nc.declare_dram_parameter("actc", [P, 2], F32, isOutput=False)
    prm["ones8"] = nc.declare_dram_parameter("ones8", [P, 8], BF16,
                                             isOutput=False)
    prm["out_t"] = nc.declare_dram_parameter("out_t", [D, L], F32, isOutput=True)
    _emit(nc, prm, repeat=repeat, phases=phases)
    nc.compile()
    _CACHE[key] = nc
    return nc


def make_in_maps(q, k, v, Wq, bq, Wk, bk, Wv, bv, Wo, bo):
    B = q.shape[0]
    f32 = np.float32
    bf16 = mybir.dt.np(BF16)

    selbc = np.zeros((P, NM, P), f32)
    for m in range(NM):
        for p in range(P):
            selbc[32 * m + (p // 64), m, p] = 1.0
    seln = np.zeros((P, NM, P), f32)
    for m in range(NM):
        for f in range(P):
            seln[f, m, 32 * m + (f // 64)] = 1.0
    actc = np.zeros((P, 2), f32)
    actc[:, 0] = 1e-24
    actc[:, 1] = LNTAUINV
    ones8 = np.ones((P, 8), f32).astype(bf16)

    in_maps = []
    for b in range(B):
        for g in range(2):
            sl = slice(g * DO, (g + 1) * DO)
            in_maps.append({
                "xq": np.ascontiguousarray(q[b].T.astype(f32)),
                "xk": np.ascontiguousarray(k[b].T.astype(f32)),
                "xv": np.ascontiguousarray(v[b].T.astype(f32)),
                "wq": np.ascontiguousarray(Wq[sl, :].T.astype(f32)),
                "wk": np.ascontiguousarray(Wk[sl, :].T.astype(f32)),
                "wv": np.ascontiguousarray(Wv[sl, :].T.astype(f32)),
                "wo": np.ascontiguousarray(Wo[:, sl].T.astype(f32)).astype(bf16),
                "bqp": np.ascontiguousarray(bq[sl].reshape(NM, P).T.astype(f32)),
                "bkp": np.ascontiguousarray(bk[sl].reshape(NM, P).T.astype(f32)),
                "seln": seln, "selbc": selbc, "actc": actc, "ones8": ones8,
            })
    return in_maps


def assemble(results, bv, Wo, bo):
    B = len(results) // 2
    bias = (bo + bv @ Wo.T).astype(np.float32)
    outs = []
    for b in range(B):
        part = (results[2 * b]["out_t"].astype(np.float32)
                + results[2 * b + 1]["out_t"].astype(np.float32))
        outs.append(part.T + bias)
    return np.stack(outs).astype(np.float32)


def kernel(q, k, v, Wq, bq, Wk, bk, Wv, bv, Wo, bo):
    q, k, v = (np.asarray(t, np.float32) for t in (q, k, v))
    Wq, bq, Wk, bk, Wv, bv, Wo, bo = (
        np.asarray(t, np.float32) for t in (Wq, bq, Wk, bk, Wv, bv, Wo, bo))
    nc = build_nc()
    in_maps = make_in_maps(q, k, v, Wq, bq, Wk, bk, Wv, bv, Wo, bo)
    last_err = None
    for attempt in range(3):
        try:
            res = run_bass_kernel_spmd(nc, in_maps, core_ids=list(range(8)))
            return assemble(res.results, bv, Wo, bo)
        except Exception as e:  # transient NRT device errors: retry
            last_err = e
            import time as _time
            _time.sleep(2.0)
    raise last_err
